# revision 1
# baseline (speedup 1.0000x reference)
"""ConvCharEmbedding Trainium2 kernel.

Reference computation (per word of C=16 chars):
    e = emb[ids]                # [C, E] gather
    y = conv1d(e.T, W, pad=2)   # [E, C], kernel K=5
    out = max_c(y + b)          # [E]

Device algorithm (per core, data-parallel over batch):
  * embedding table cast to bf16, laid out in SBUF as
    [128 partitions, 3 ranks * 128] (row v at partition v%128, rank v//128),
    plus a zero row at index 262 used for conv padding.
  * char ids padded to 20 slots/word (2 zero-row slots each side) so conv
    windows never cross word boundaries; dma_gather (transpose mode) writes
    gathered embeddings directly as e^T [E=128 partitions, slot].
  * conv = 5 PSUM-accumulated matmuls; lhsT = W_k^T [i, o] (bf16), rhs =
    shifted windows of e^T, 512 output positions (32 words x 16 chars) per
    PSUM group.
  * DVE max-reduce over the 16 char positions straight out of PSUM,
    ACT bias add, PE transpose [o, n] -> [n, o], DMA out.
"""

import numpy as np

import concourse.bass as bass
import concourse.tile as tile
from concourse import bacc, mybir
from concourse.bass_utils import run_bass_kernel_spmd
from concourse.masks import make_identity

B, W, C = 128, 256, 16
E = 128
K = 5
PAD = 2
V = 262
VZERO = 262  # zero row index in the padded table
NCORES = 8
B_SH = B // NCORES            # 16 batches per core
N = B_SH * W                  # 4096 words per core
SLOTS = C + 2 * PAD           # 20 slots per word (padded)
CHUNK_N = 512                 # words per gather chunk
NCHUNK = N // CHUNK_N         # 8
IDX_CHUNK = CHUNK_N * SLOTS   # 10240 gather indices per chunk
NQUEUES = 4                   # SWDGE queues to round-robin gathers over
GROUP_N = 32                  # words per PSUM group (32*16 = 512 positions)
NGROUP = CHUNK_N // GROUP_N   # 32 groups per chunk
TILES_PER_CHUNK = CHUNK_N // 128  # 8 output transpose tiles per chunk

dt = mybir.dt


def build_program():
    nc = bacc.Bacc("TRN2", target_bir_lowering=False, debug=False,
                   num_devices=NCORES, dynamic_dma_scratch_size=16384)

    ids_d = nc.dram_tensor("ids", [128, N * SLOTS // 128], dt.int32,
                           kind="ExternalInput")
    ident_d = nc.dram_tensor("ident", [128, 128], dt.float32,
                             kind="ExternalInput")
    emb_d = nc.dram_tensor("emb", [V, E], dt.float32, kind="ExternalInput")
    convw_d = nc.dram_tensor("convw", [E, E * K], dt.float32,
                             kind="ExternalInput")
    convb_d = nc.dram_tensor("convb", [E, 1], dt.float32, kind="ExternalInput")
    out_d = nc.dram_tensor("out", [N, E], dt.float32, kind="ExternalOutput")
    emb_tab = nc.dram_tensor("embtab", [264, E], dt.float32)

    with tile.TileContext(nc) as tc:
        with (
            tc.tile_pool(name="const", bufs=1) as const_pool,
            tc.tile_pool(name="stage", bufs=2) as stage_pool,
            tc.tile_pool(name="et", bufs=2) as et_pool,
            tc.tile_pool(name="gth", bufs=8) as gpool,
            tc.tile_pool(name="yc", bufs=2) as y_pool,
            tc.tile_pool(name="osb", bufs=2) as out_pool,
            tc.tile_pool(name="ps", bufs=4, space="PSUM") as ps_pool,
            tc.tile_pool(name="pst", bufs=2, space="PSUM") as pst_pool,
        ):
            # ---- prologue: weights, table, bias, indices ----
            ident = const_pool.tile([128, 128], dt.float32)
            nc.sync.dma_start(ident[:], ident_d.ap())

            conv_sb = const_pool.tile([E, E * K], dt.float32)
            nc.sync.dma_start(conv_sb[:], convw_d.ap())
            conv_kview = conv_sb[:].rearrange("o (i k) -> o k i", k=K)

            wt = const_pool.tile([E, K, E], dt.float32r)  # W_k^T, fp32r
            for k in range(K):
                pt = pst_pool.tile([128, 128], dt.float32)
                nc.tensor.transpose(pt[:], conv_kview[:, k, :], ident[:])
                nc.vector.tensor_copy(wt[:, k, :], pt[:])

            # f32 table in DRAM: rows 0..261 = emb, rows 262+ = zeros (pad)
            for r in range(2):
                es = stage_pool.tile([128, E], dt.float32, tag="embstage")
                nc.sync.dma_start(es[:], emb_d.ap()[r * 128:(r + 1) * 128, :])
                nc.sync.dma_start(emb_tab.ap()[r * 128:(r + 1) * 128, :], es[:])
            et_tail = stage_pool.tile([128, E], dt.float32, tag="embstage")
            nc.vector.memset(et_tail[:], 0)
            nc.sync.dma_start(et_tail[0:V - 256, :], emb_d.ap()[256:V, :])
            nc.sync.dma_start(emb_tab.ap()[256:264, :], et_tail[0:8, :])

            bias = const_pool.tile([E, 1], dt.float32)
            nc.sync.dma_start(bias[:], convb_d.ap())

            idx_sb = const_pool.tile([128, N * SLOTS // 128], dt.int32)
            nc.sync.dma_start(idx_sb[:], ids_d.ap())

            out_view = out_d.ap().rearrange("(t p) o -> p t o", p=128)

            # ---- main loop ----
            TPC = IDX_CHUNK // 128  # gather tiles per chunk (80)
            for c in range(NCHUNK):
                et = et_pool.tile([128, 1, IDX_CHUNK], dt.float32r)
                for t in range(TPC):
                    tg = c * TPC + t
                    g = gpool.tile([128, 128], dt.float32)
                    nc.gpsimd.indirect_dma_start(
                        out=g[:], out_offset=None, in_=emb_tab.ap(),
                        in_offset=bass.IndirectOffsetOnAxis(
                            ap=idx_sb[:, tg:tg + 1], axis=0))
                    gp = pst_pool.tile([128, 128], dt.float32)
                    nc.tensor.transpose(gp[:], g[:], ident[:])
                    nc.scalar.copy(et[:, 0, t * 128:(t + 1) * 128], gp[:])
                win = et[:, 0, :].rearrange("p (n s) -> p n s", s=SLOTS)

                y_c = y_pool.tile([128, CHUNK_N], dt.float32)
                for g in range(NGROUP):
                    ps = ps_pool.tile([128, 512], dt.float32)
                    for k in range(K):
                        rhs = win[:, g * GROUP_N:(g + 1) * GROUP_N, k:k + C]
                        nc.tensor.matmul(ps[:], lhsT=wt[:, k, :], rhs=rhs,
                                         start=(k == 0), stop=(k == K - 1))
                    nc.vector.tensor_reduce(
                        out=y_c[:, g * GROUP_N:(g + 1) * GROUP_N],
                        in_=ps[:].rearrange("p (n c) -> p n c", c=C),
                        axis=mybir.AxisListType.X,
                        op=mybir.AluOpType.max,
                    )
                nc.scalar.add(y_c[:], y_c[:], bias[:, 0:1])

                osb = out_pool.tile([128, TILES_PER_CHUNK, 128], dt.float32)
                for tl in range(TILES_PER_CHUNK):
                    pt = pst_pool.tile([128, 128], dt.float32)
                    nc.tensor.transpose(pt[:], y_c[:, tl * 128:(tl + 1) * 128],
                                        ident[:])
                    nc.scalar.copy(osb[:, tl, :], pt[:])
                nc.sync.dma_start(
                    out_view[:, c * TILES_PER_CHUNK:(c + 1) * TILES_PER_CHUNK, :],
                    osb[:])

    nc.compile()
    return nc


def prep_core_inputs(ids_core: np.ndarray, emb_weight: np.ndarray,
                     conv_w: np.ndarray, conv_b: np.ndarray) -> dict:
    """ids_core: [B_SH, W, C] int64 for this core."""
    ids = ids_core.reshape(N, C).astype(np.int32)
    padded = np.full((N, SLOTS), VZERO, dtype=np.int32)
    padded[:, PAD:PAD + C] = ids
    flat = padded.reshape(-1)
    idx32 = flat.reshape(-1, 128).T.copy()           # [128, N*SLOTS/128]
    return {
        "ids": idx32,
        "ident": np.eye(128, dtype=np.float32),
        "emb": np.ascontiguousarray(emb_weight, dtype=np.float32),
        "convw": np.ascontiguousarray(
            conv_w.astype(np.float32).reshape(E, E * K)),
        "convb": np.ascontiguousarray(
            conv_b.astype(np.float32).reshape(E, 1)),
    }


_prog_cache = {}


def kernel(input, lengths, emb_weight, conv_w, conv_b, _trace=False):
    input = np.asarray(input)
    emb_weight = np.asarray(emb_weight, dtype=np.float32)
    conv_w = np.asarray(conv_w, dtype=np.float32)
    conv_b = np.asarray(conv_b, dtype=np.float32)

    if "nc" not in _prog_cache:
        _prog_cache["nc"] = build_program()
    nc = _prog_cache["nc"]

    core_ids = list(range(NCORES))
    in_maps = [
        prep_core_inputs(input[i * B_SH:(i + 1) * B_SH], emb_weight, conv_w,
                         conv_b)
        for i in core_ids
    ]
    res = run_bass_kernel_spmd(nc, in_maps, core_ids, trace=_trace)
    out = np.concatenate([res.results[i]["out"] for i in core_ids], axis=0)
    out = out.reshape(B, W, E).astype(np.float32)
    if _trace:
        kernel.last_exec_time_ns = res.exec_time_ns
        kernel.last_results = res
    return out

